# revision 31
# baseline (speedup 1.0000x reference)
"""IoU loss kernel for Trainium2, data-parallel over the batch dim on 8 cores.

Math (per reference):
    probs = softmax(inputs, axis=1)                       # (8, 13, 800, 800)
    intersection = sum_pix probs[b, t, h, w]
    total = probs.sum() + Npix                            # probs.sum() == Npix (+fp noise)
    out = 1 - (intersection + smooth) / (total - intersection + smooth)

Device kernel (per core, one batch item), raw Bass with manual semaphores.
Layout: pixel-partitioned chunks with classes PAIR-PACKED: per pixel the
12 even classes are stored as 6 f32 "elements" (two adjacent bf16 each),
class 12 dense at the tail. copy_predicated is element-rate-capped, so
muxing 6 f32 pairs (5 element-selects) + 2 bf16 fix-ups touches 7N
elements instead of 12N for the plain class tree.

Per chunk j:
  ACT : E = exp(X)  (one dense bf16 op over all 13N values)
  DVE : pair mux tree on the f32 view of X, in place =>
        winner pair in f32 slot 0; then lo<-hi if t odd, lo<-x12 if t==12
  DVE : denominator tree on flat E slices (dense bf16 adds + pair fold)
  ACT : L = ln(C)
  DVE : S = x_sel - L
  ACT : exp(S) with accum_out -> acc[:, j]
Host sums acc over cores/partitions/chunks and forms the IoU scalar.
"""

import numpy as np
import ml_dtypes

_BS, _C, _H, _W = 8, 13, 800, 800
_P = 128
_FREE = (_H * _W) // _P  # 5000
_N = 500                 # chunk free size (pixels)
_NCHUNK = _FREE // _N    # 10
_NBUF = 4
_NCORES = 8
_NPIX = _BS * _H * _W    # 5120000
_FLAT = 13 * _N          # 6500 bf16 per chunk per partition

_cached = {}


def _build_program():
    from contextlib import ExitStack

    import concourse.bass as bass
    import concourse.mybir as mybir

    f32 = mybir.dt.float32
    bf16 = mybir.dt.bfloat16
    u8 = mybir.dt.uint8
    Alu = mybir.AluOpType
    Act = mybir.ActivationFunctionType

    nc = bass.Bass(trn_type="TRN2")
    x = nc.declare_dram_parameter("x", [_P, _NCHUNK, _FLAT], bf16,
                                  isOutput=False)
    m = nc.declare_dram_parameter("m", [_P, 5, _FREE], u8, isOutput=False)
    part = nc.declare_dram_parameter("part", [_P, _NCHUNK], f32, isOutput=True)

    ctx = ExitStack()
    with ctx:
        M = ctx.enter_context(nc.sbuf_tensor("M", [_P, 5, _FREE], u8))
        X = ctx.enter_context(nc.sbuf_tensor("X", [_P, _NBUF, _FLAT], bf16))
        E = ctx.enter_context(nc.sbuf_tensor("E", [_P, _NBUF, _FLAT], bf16))
        A = ctx.enter_context(nc.sbuf_tensor("A", [_P, 6 * _N], bf16))
        B = ctx.enter_context(nc.sbuf_tensor("B", [_P, 2 * _N], bf16))
        C2 = ctx.enter_context(nc.sbuf_tensor("C2", [_P, 2 * _N], bf16))
        C1 = ctx.enter_context(nc.sbuf_tensor("C1", [_P, _N], bf16))
        CD = ctx.enter_context(nc.sbuf_tensor("CD", [_P, _NBUF, _N], bf16))
        L = ctx.enter_context(nc.sbuf_tensor("L", [_P, _NBUF, _N], bf16))
        S = ctx.enter_context(nc.sbuf_tensor("S", [_P, _NBUF, _N], bf16))
        ED = ctx.enter_context(nc.sbuf_tensor("ED", [_P, _N], bf16))
        acc = ctx.enter_context(nc.sbuf_tensor("acc", [_P, _NCHUNK], f32))

        block = ctx.enter_context(nc.Block())
        dma_m = ctx.enter_context(nc.semaphore("dma_m"))
        dma_x0a = ctx.enter_context(nc.semaphore("dma_x0a"))
        s_e0a = ctx.enter_context(nc.semaphore("s_e0a"))
        dma_xc = [ctx.enter_context(nc.semaphore(f"dma_xc{i}"))
                  for i in range(_NBUF)]
        dma_out = ctx.enter_context(nc.semaphore("dma_out"))
        s_exp = ctx.enter_context(nc.semaphore("s_exp"))
        s_C = ctx.enter_context(nc.semaphore("s_C"))
        s_ln = ctx.enter_context(nc.semaphore("s_ln"))
        s_sub = ctx.enter_context(nc.semaphore("s_sub"))
        s_fin = ctx.enter_context(nc.semaphore("s_fin"))

        HM = _FREE // 2

        @block.sync
        def _(sync):
            # chunk 0 in two pieces so exp and the pair tree start early
            sync.dma_start(out=X[:, 0, 0:3000],
                           in_=x[:, 0, 0:3000]).then_inc(dma_x0a, 16)
            sync.dma_start(out=M[:, :, 0:HM],
                           in_=m[:, :, 0:HM]).then_inc(dma_m, 16)
            sync.dma_start(out=X[:, 0, 3000:_FLAT],
                           in_=x[:, 0, 3000:_FLAT]).then_inc(dma_xc[0], 16)
            for j in range(1, _NCHUNK):
                b = j % _NBUF
                if j >= _NBUF:
                    # X slot b last read by sub of chunk j-NBUF
                    sync.wait_ge(s_sub, j - _NBUF + 1)
                    sync.wait_ge(dma_xc[b], 16 * (j // _NBUF))
                sync.dma_start(
                    out=X[:, b, :], in_=x[:, j, :]
                ).then_inc(dma_xc[b], 16)
                if j == 2:
                    sync.dma_start(out=M[:, :, HM:_FREE],
                                   in_=m[:, :, HM:_FREE]).then_inc(dma_m, 16)
            sync.wait_ge(s_fin, _NCHUNK)
            sync.dma_start(out=part[:, :], in_=acc[:, :]).then_inc(dma_out, 16)
            sync.wait_ge(dma_out, 16)

        @block.scalar
        def _(scalar):
            def ln_of(k):
                bk = k % _NBUF
                scalar.wait_ge(s_C, k + 1)
                if k >= _NBUF:
                    scalar.wait_ge(s_sub, k - _NBUF + 1)
                scalar.activation(
                    out=L[:, bk, :], in_=CD[:, bk, :], func=Act.Ln
                ).then_inc(s_ln, 1)

            def expacc_of(k):
                bk = k % _NBUF
                scalar.wait_ge(s_sub, k + 1)
                scalar.activation(
                    out=ED[:, :], in_=S[:, bk, :], func=Act.Exp,
                    accum_out=acc[:, k:k + 1],
                ).then_inc(s_fin, 1)

            # dummy activation to preload the ACT table
            scalar.activation(out=ED[:, 0:1], in_=ED[:, 0:1], func=Act.Exp)
            for j in range(_NCHUNK):
                b = j % _NBUF
                if j == 0:
                    scalar.wait_ge(dma_x0a, 16)
                    scalar.activation(
                        out=E[:, 0, 0:3000], in_=X[:, 0, 0:3000],
                        func=Act.Exp,
                    ).then_inc(s_e0a, 1)
                    scalar.wait_ge(dma_xc[0], 16)
                    scalar.activation(
                        out=E[:, 0, 3000:_FLAT], in_=X[:, 0, 3000:_FLAT],
                        func=Act.Exp,
                    ).then_inc(s_exp, 1)
                    continue
                scalar.wait_ge(dma_xc[b], 16 * (j // _NBUF + 1))
                if j >= _NBUF:
                    # E slot fully consumed by the D tree of chunk j-NBUF
                    scalar.wait_ge(s_C, j - _NBUF + 1)
                scalar.activation(
                    out=E[:, b, :], in_=X[:, b, :], func=Act.Exp
                ).then_inc(s_exp, 1)
                if j >= 1:
                    ln_of(j - 1)
                if j >= 2:
                    expacc_of(j - 2)
            ln_of(_NCHUNK - 1)
            expacc_of(_NCHUNK - 2)
            expacc_of(_NCHUNK - 1)

        @block.vector
        def _(vector):
            def sub_of(k):
                bk = k % _NBUF
                vector.wait_ge(s_ln, k + 1)
                if k >= _NBUF:
                    vector.wait_ge(s_fin, k - _NBUF + 1)
                lo = X[:, bk, 0:2 * _N].rearrange("p (n k) -> p n k", k=2)
                vector.tensor_tensor(
                    out=S[:, bk, :].unsqueeze(2), in0=lo[:, :, 0:1],
                    in1=L[:, bk, :].unsqueeze(2), op=Alu.subtract,
                ).then_inc(s_sub, 1)

            vector.wait_ge(dma_m, 16)
            NH = _N // 2
            for j in range(_NCHUNK):
                b = j % _NBUF
                if j == _NCHUNK // 2:
                    vector.wait_ge(dma_m, 32)
                if j == 0:
                    # the tree writes X bytes [0:3000] (read-done after the
                    # first exp piece) and reads raw logits beyond (loaded,
                    # not yet exp'd)
                    vector.wait_ge(s_e0a, 1)
                    vector.wait_ge(dma_xc[0], 16)
                else:
                    vector.wait_ge(s_exp, j + 1)
                XB = X[:, b, :]
                XF = XB[:, 0:6000].bitcast(f32)  # (128, 3000): 6 pair items
                XP = XB[:, 0:2 * _N].rearrange("p (n k) -> p n k", k=2)
                # pair mux tree, levels split in free-dim halves to hide
                # the predicated-write drain between dependent levels
                for h in range(2):
                    isl = slice(h * NH, (h + 1) * NH)
                    msl = slice(j * _N + h * NH, j * _N + (h + 1) * NH)
                    mk = M[:, 0, msl].unsqueeze(1)
                    vector.copy_predicated(
                        XF[:, 0:3 * _N].rearrange(
                            "p (c n) -> p c n", c=3)[:, :, isl],
                        mk.broadcast_to((_P, 3, NH)),
                        XF[:, 3 * _N:6 * _N].rearrange(
                            "p (c n) -> p c n", c=3)[:, :, isl])
                for lev, off in ((1, _N), (2, 2 * _N)):
                    for h in range(2):
                        isl = slice(h * NH, (h + 1) * NH)
                        msl = slice(j * _N + h * NH, j * _N + (h + 1) * NH)
                        vector.copy_predicated(
                            XF[:, 0:_N][:, isl], M[:, lev, msl],
                            XF[:, off:off + _N][:, isl])
                # unpack: lo <- hi where t odd; lo <- x12 where t == 12
                for h in range(2):
                    isl = slice(h * NH, (h + 1) * NH)
                    msl = slice(j * _N + h * NH, j * _N + (h + 1) * NH)
                    vector.copy_predicated(
                        XP[:, isl, 0:1], M[:, 3, msl].unsqueeze(2),
                        XP[:, isl, 1:2])
                    vector.copy_predicated(
                        XP[:, isl, 0:1], M[:, 4, msl].unsqueeze(2),
                        XB[:, 6000:6500][:, isl].unsqueeze(2))
                # denominator tree on flat E slices
                EB = E[:, b, :]
                if j == 0:
                    vector.wait_ge(s_exp, 1)
                vector.tensor_tensor(out=A[:, :], in0=EB[:, 0:3000],
                                     in1=EB[:, 3000:6000], op=Alu.add)
                if j >= _NBUF:
                    vector.wait_ge(s_ln, j - _NBUF + 1)
                vector.tensor_tensor(out=B[:, :], in0=A[:, 0:1000],
                                     in1=A[:, 1000:2000], op=Alu.add)
                vector.tensor_tensor(out=C2[:, :], in0=B[:, :],
                                     in1=A[:, 2000:3000], op=Alu.add)
                C2v = C2[:, :].rearrange("p (n k) -> p n k", k=2)
                vector.tensor_tensor(out=C1[:, :].unsqueeze(2),
                                     in0=C2v[:, :, 0:1], in1=C2v[:, :, 1:2],
                                     op=Alu.add)
                vector.tensor_tensor(
                    out=CD[:, b, :], in0=C1[:, :], in1=EB[:, 6000:6500],
                    op=Alu.add,
                ).then_inc(s_C, 1)
                if j >= 1:
                    sub_of(j - 1)
            sub_of(_NCHUNK - 1)

    return nc


def _get_program():
    if "nc" not in _cached:
        _cached["nc"] = _build_program()
    return _cached["nc"]


def _make_in_maps(inputs, targets):
    in_maps = []
    for bb in range(_NCORES):
        xb = np.asarray(inputs[bb]).reshape(_C, _P, _FREE)
        t0 = np.ascontiguousarray(xb.transpose(1, 0, 2)).astype(
            ml_dtypes.bfloat16).reshape(_P, _C, _NCHUNK, _N)
        # pairs: (part, chunk, pairidx, i, k) flattened to 6000 bf16,
        # then class 12 dense (500)
        pairs = t0[:, 0:12].reshape(_P, 6, 2, _NCHUNK, _N).transpose(
            0, 3, 1, 4, 2).reshape(_P, _NCHUNK, 6000)
        c12 = t0[:, 12].transpose(0, 1, 2).reshape(_P, _NCHUNK, _N)
        xh = np.ascontiguousarray(np.concatenate([pairs, c12], axis=2))
        t = np.asarray(targets[bb]).astype(np.int64).reshape(_P, _FREE)
        tp = np.minimum(t, 11) // 2
        m1 = tp >= 3
        t1p = tp - 3 * m1
        m2a = t1p == 1
        m2b = t1p == 2
        m3 = (t % 2 == 1) & (t < 12)
        m4 = t == 12
        mh = np.ascontiguousarray(
            np.stack([m1, m2a, m2b, m3, m4], axis=1).astype(np.uint8))
        in_maps.append({"x": xh, "m": mh})
    return in_maps


def _finalize(parts, smooth):
    inter = 0.0
    for p in parts:
        inter += float(np.sum(np.asarray(p).astype(np.float64)))
    s = float(smooth)
    total = 2.0 * float(_NPIX)
    union = total - inter
    out = 1.0 - (inter + s) / (union + s)
    return np.asarray(np.float32(out))


def kernel(inputs, targets, smooth):
    from concourse.bass_utils import run_bass_kernel_spmd

    nc = _get_program()
    in_maps = _make_in_maps(np.asarray(inputs), np.asarray(targets))
    res = run_bass_kernel_spmd(nc, in_maps, list(range(_NCORES)))
    return _finalize([res.results[bb]["part"] for bb in range(_NCORES)], smooth)


# revision 32
# speedup vs baseline: 1.0002x; 1.0002x over previous
"""IoU loss kernel for Trainium2, data-parallel over the batch dim on 8 cores.

Math (per reference):
    probs = softmax(inputs, axis=1)                       # (8, 13, 800, 800)
    intersection = sum_pix probs[b, t, h, w]
    total = probs.sum() + Npix                            # probs.sum() == Npix (+fp noise)
    out = 1 - (intersection + smooth) / (total - intersection + smooth)

Device kernel (per core, one batch item), raw Bass with manual semaphores.
Layout: pixel-partitioned chunks with classes PAIR-PACKED: per pixel the
12 even classes are stored as 6 f32 "elements" (two adjacent bf16 each),
class 12 dense at the tail. copy_predicated is element-rate-capped, so
muxing 6 f32 pairs (5 element-selects) + 2 bf16 fix-ups touches 7N
elements instead of 12N for the plain class tree.

Per chunk j:
  ACT : E = exp(X)  (one dense bf16 op over all 13N values)
  DVE : pair mux tree on the f32 view of X, in place =>
        winner pair in f32 slot 0; then lo<-hi if t odd, lo<-x12 if t==12
  DVE : denominator tree on flat E slices (dense bf16 adds + pair fold)
  ACT : L = ln(C)
  DVE : S = x_sel - L
  ACT : exp(S) with accum_out -> acc[:, j]
Host sums acc over cores/partitions/chunks and forms the IoU scalar.
"""

import numpy as np
import ml_dtypes

_BS, _C, _H, _W = 8, 13, 800, 800
_P = 128
_FREE = (_H * _W) // _P  # 5000
_N = 500                 # chunk free size (pixels)
_NCHUNK = _FREE // _N    # 10
_NBUF = 4
_NCORES = 8
_NPIX = _BS * _H * _W    # 5120000
_FLAT = 13 * _N          # 6500 bf16 per chunk per partition

_cached = {}


def _build_program():
    from contextlib import ExitStack

    import concourse.bass as bass
    import concourse.mybir as mybir

    f32 = mybir.dt.float32
    bf16 = mybir.dt.bfloat16
    u8 = mybir.dt.uint8
    Alu = mybir.AluOpType
    Act = mybir.ActivationFunctionType

    nc = bass.Bass(trn_type="TRN2")
    x = nc.declare_dram_parameter("x", [_P, _NCHUNK, _FLAT], bf16,
                                  isOutput=False)
    m = nc.declare_dram_parameter("m", [_P, 5, _FREE], u8, isOutput=False)
    part = nc.declare_dram_parameter("part", [_P, _NCHUNK], f32, isOutput=True)

    ctx = ExitStack()
    with ctx:
        M = ctx.enter_context(nc.sbuf_tensor("M", [_P, 5, _FREE], u8))
        X = ctx.enter_context(nc.sbuf_tensor("X", [_P, _NBUF, _FLAT], bf16))
        E = ctx.enter_context(nc.sbuf_tensor("E", [_P, _NBUF, _FLAT], bf16))
        A = ctx.enter_context(nc.sbuf_tensor("A", [_P, 6 * _N], bf16))
        B = ctx.enter_context(nc.sbuf_tensor("B", [_P, 2 * _N], bf16))
        C2 = ctx.enter_context(nc.sbuf_tensor("C2", [_P, 2 * _N], bf16))
        C1 = ctx.enter_context(nc.sbuf_tensor("C1", [_P, _N], bf16))
        CD = ctx.enter_context(nc.sbuf_tensor("CD", [_P, _NBUF, _N], bf16))
        L = ctx.enter_context(nc.sbuf_tensor("L", [_P, _NBUF, _N], bf16))
        S = ctx.enter_context(nc.sbuf_tensor("S", [_P, _NBUF, _N], bf16))
        ED = ctx.enter_context(nc.sbuf_tensor("ED", [_P, _N], bf16))
        acc = ctx.enter_context(nc.sbuf_tensor("acc", [_P, _NCHUNK], f32))

        block = ctx.enter_context(nc.Block())
        dma_m = ctx.enter_context(nc.semaphore("dma_m"))
        dma_x0a = ctx.enter_context(nc.semaphore("dma_x0a"))
        s_e0a = ctx.enter_context(nc.semaphore("s_e0a"))
        dma_xc = [ctx.enter_context(nc.semaphore(f"dma_xc{i}"))
                  for i in range(_NBUF)]
        dma_out = ctx.enter_context(nc.semaphore("dma_out"))
        s_exp = ctx.enter_context(nc.semaphore("s_exp"))
        s_C = ctx.enter_context(nc.semaphore("s_C"))
        s_ln = ctx.enter_context(nc.semaphore("s_ln"))
        s_sub = ctx.enter_context(nc.semaphore("s_sub"))
        s_fin = ctx.enter_context(nc.semaphore("s_fin"))

        HM = _FREE // 2

        @block.sync
        def _(sync):
            # chunk 0 in two pieces so exp and the pair tree start early
            sync.dma_start(out=X[:, 0, 0:3000],
                           in_=x[:, 0, 0:3000]).then_inc(dma_x0a, 16)
            sync.dma_start(out=M[:, :, 0:HM],
                           in_=m[:, :, 0:HM]).then_inc(dma_m, 16)
            sync.dma_start(out=X[:, 0, 3000:_FLAT],
                           in_=x[:, 0, 3000:_FLAT]).then_inc(dma_xc[0], 16)
            for j in range(1, _NCHUNK):
                b = j % _NBUF
                if j >= _NBUF:
                    # X slot b last read by sub of chunk j-NBUF
                    sync.wait_ge(s_sub, j - _NBUF + 1)
                    sync.wait_ge(dma_xc[b], 16 * (j // _NBUF))
                sync.dma_start(
                    out=X[:, b, :], in_=x[:, j, :]
                ).then_inc(dma_xc[b], 16)
                if j == 2:
                    sync.dma_start(out=M[:, :, HM:_FREE],
                                   in_=m[:, :, HM:_FREE]).then_inc(dma_m, 16)
            sync.wait_ge(s_fin, _NCHUNK)
            sync.dma_start(out=part[:, :], in_=acc[:, :]).then_inc(dma_out, 16)
            sync.wait_ge(dma_out, 16)

        @block.scalar
        def _(scalar):
            def ln_of(k):
                bk = k % _NBUF
                scalar.wait_ge(s_C, k + 1)
                if k >= _NBUF:
                    scalar.wait_ge(s_sub, k - _NBUF + 1)
                scalar.activation(
                    out=L[:, bk, :], in_=CD[:, bk, :], func=Act.Ln
                ).then_inc(s_ln, 1)

            def expacc_of(k):
                bk = k % _NBUF
                scalar.wait_ge(s_sub, k + 1)
                scalar.activation(
                    out=ED[:, :], in_=S[:, bk, :], func=Act.Exp,
                    accum_out=acc[:, k:k + 1],
                ).then_inc(s_fin, 1)

            # dummy activation to preload the ACT table
            scalar.activation(out=ED[:, 0:1], in_=ED[:, 0:1], func=Act.Exp)
            for j in range(_NCHUNK):
                b = j % _NBUF
                if j == 0:
                    scalar.wait_ge(dma_x0a, 16)
                    scalar.activation(
                        out=E[:, 0, 0:3000], in_=X[:, 0, 0:3000],
                        func=Act.Exp,
                    ).then_inc(s_e0a, 1)
                    scalar.wait_ge(dma_xc[0], 16)
                    scalar.activation(
                        out=E[:, 0, 3000:_FLAT], in_=X[:, 0, 3000:_FLAT],
                        func=Act.Exp,
                    ).then_inc(s_exp, 1)
                    continue
                scalar.wait_ge(dma_xc[b], 16 * (j // _NBUF + 1))
                if j >= _NBUF:
                    # E slot fully consumed by the D tree of chunk j-NBUF
                    scalar.wait_ge(s_C, j - _NBUF + 1)
                scalar.activation(
                    out=E[:, b, :], in_=X[:, b, :], func=Act.Exp
                ).then_inc(s_exp, 1)
                if j >= 1:
                    ln_of(j - 1)
                if j >= 2:
                    expacc_of(j - 2)
            ln_of(_NCHUNK - 1)
            expacc_of(_NCHUNK - 2)
            expacc_of(_NCHUNK - 1)

        @block.vector
        def _(vector):
            def sub_of(k):
                bk = k % _NBUF
                vector.wait_ge(s_ln, k + 1)
                if k >= _NBUF:
                    vector.wait_ge(s_fin, k - _NBUF + 1)
                lo = X[:, bk, 0:2 * _N].rearrange("p (n k) -> p n k", k=2)
                vector.tensor_tensor(
                    out=S[:, bk, :].unsqueeze(2), in0=lo[:, :, 0:1],
                    in1=L[:, bk, :].unsqueeze(2), op=Alu.subtract,
                ).then_inc(s_sub, 1)

            vector.wait_ge(dma_m, 16)
            NH = _N // 2
            for j in range(_NCHUNK):
                b = j % _NBUF
                if j == _NCHUNK // 2:
                    vector.wait_ge(dma_m, 32)
                if j == 0:
                    # the tree writes X bytes [0:3000] (read-done after the
                    # first exp piece) and reads raw logits beyond (loaded,
                    # not yet exp'd)
                    vector.wait_ge(s_e0a, 1)
                    vector.wait_ge(dma_xc[0], 16)
                else:
                    vector.wait_ge(s_exp, j + 1)
                # denominator tree on flat E slices
                EB = E[:, b, :]
                if j == 0:
                    vector.wait_ge(s_exp, 1)
                vector.tensor_tensor(out=A[:, :], in0=EB[:, 0:3000],
                                     in1=EB[:, 3000:6000], op=Alu.add)
                if j >= _NBUF:
                    vector.wait_ge(s_ln, j - _NBUF + 1)
                vector.tensor_tensor(out=B[:, :], in0=A[:, 0:1000],
                                     in1=A[:, 1000:2000], op=Alu.add)
                vector.tensor_tensor(out=C2[:, :], in0=B[:, :],
                                     in1=A[:, 2000:3000], op=Alu.add)
                C2v = C2[:, :].rearrange("p (n k) -> p n k", k=2)
                vector.tensor_tensor(out=C1[:, :].unsqueeze(2),
                                     in0=C2v[:, :, 0:1], in1=C2v[:, :, 1:2],
                                     op=Alu.add)
                vector.tensor_tensor(
                    out=CD[:, b, :], in0=C1[:, :], in1=EB[:, 6000:6500],
                    op=Alu.add,
                ).then_inc(s_C, 1)
                XB = X[:, b, :]
                XF = XB[:, 0:6000].bitcast(f32)  # (128, 3000): 6 pair items
                XP = XB[:, 0:2 * _N].rearrange("p (n k) -> p n k", k=2)
                # pair mux tree, levels split in free-dim halves to hide
                # the predicated-write drain between dependent levels
                for h in range(2):
                    isl = slice(h * NH, (h + 1) * NH)
                    msl = slice(j * _N + h * NH, j * _N + (h + 1) * NH)
                    mk = M[:, 0, msl].unsqueeze(1)
                    vector.copy_predicated(
                        XF[:, 0:3 * _N].rearrange(
                            "p (c n) -> p c n", c=3)[:, :, isl],
                        mk.broadcast_to((_P, 3, NH)),
                        XF[:, 3 * _N:6 * _N].rearrange(
                            "p (c n) -> p c n", c=3)[:, :, isl])
                for lev, off in ((1, _N), (2, 2 * _N)):
                    for h in range(2):
                        isl = slice(h * NH, (h + 1) * NH)
                        msl = slice(j * _N + h * NH, j * _N + (h + 1) * NH)
                        vector.copy_predicated(
                            XF[:, 0:_N][:, isl], M[:, lev, msl],
                            XF[:, off:off + _N][:, isl])
                # unpack: lo <- hi where t odd; lo <- x12 where t == 12
                for h in range(2):
                    isl = slice(h * NH, (h + 1) * NH)
                    msl = slice(j * _N + h * NH, j * _N + (h + 1) * NH)
                    vector.copy_predicated(
                        XP[:, isl, 0:1], M[:, 3, msl].unsqueeze(2),
                        XP[:, isl, 1:2])
                    vector.copy_predicated(
                        XP[:, isl, 0:1], M[:, 4, msl].unsqueeze(2),
                        XB[:, 6000:6500][:, isl].unsqueeze(2))
                if j >= 1:
                    sub_of(j - 1)
            sub_of(_NCHUNK - 1)

    return nc


def _get_program():
    if "nc" not in _cached:
        _cached["nc"] = _build_program()
    return _cached["nc"]


def _make_in_maps(inputs, targets):
    in_maps = []
    for bb in range(_NCORES):
        xb = np.asarray(inputs[bb]).reshape(_C, _P, _FREE)
        t0 = np.ascontiguousarray(xb.transpose(1, 0, 2)).astype(
            ml_dtypes.bfloat16).reshape(_P, _C, _NCHUNK, _N)
        # pairs: (part, chunk, pairidx, i, k) flattened to 6000 bf16,
        # then class 12 dense (500)
        pairs = t0[:, 0:12].reshape(_P, 6, 2, _NCHUNK, _N).transpose(
            0, 3, 1, 4, 2).reshape(_P, _NCHUNK, 6000)
        c12 = t0[:, 12].transpose(0, 1, 2).reshape(_P, _NCHUNK, _N)
        xh = np.ascontiguousarray(np.concatenate([pairs, c12], axis=2))
        t = np.asarray(targets[bb]).astype(np.int64).reshape(_P, _FREE)
        tp = np.minimum(t, 11) // 2
        m1 = tp >= 3
        t1p = tp - 3 * m1
        m2a = t1p == 1
        m2b = t1p == 2
        m3 = (t % 2 == 1) & (t < 12)
        m4 = t == 12
        mh = np.ascontiguousarray(
            np.stack([m1, m2a, m2b, m3, m4], axis=1).astype(np.uint8))
        in_maps.append({"x": xh, "m": mh})
    return in_maps


def _finalize(parts, smooth):
    inter = 0.0
    for p in parts:
        inter += float(np.sum(np.asarray(p).astype(np.float64)))
    s = float(smooth)
    total = 2.0 * float(_NPIX)
    union = total - inter
    out = 1.0 - (inter + s) / (union + s)
    return np.asarray(np.float32(out))


def kernel(inputs, targets, smooth):
    from concourse.bass_utils import run_bass_kernel_spmd

    nc = _get_program()
    in_maps = _make_in_maps(np.asarray(inputs), np.asarray(targets))
    res = run_bass_kernel_spmd(nc, in_maps, list(range(_NCORES)))
    return _finalize([res.results[bb]["part"] for bb in range(_NCORES)], smooth)
